# revision 10
# baseline (speedup 1.0000x reference)
"""Self-contained Trainium2 Bass kernel for MultiHeadAttention.

Problem: B=2, S=2048, D=1024, H=16, hd=64, with the reference's
masked_fill(mask==0, -1e-09) quirk: masked scores become ~0.0, so
exp(masked) == 1.0 in fp32 and every key position participates in the
softmax denominator. Fully-masked key blocks therefore contribute a
block-constant suffix sum of V rows, added via cheap rank-1-style
matmuls instead of full score/attn matmuls.

Sharding: 8 cores = 2 batches x 4 head-groups (4 heads per core).
Each core computes a partial [S, D] output (its 4 heads pushed through
the O-projection); the host sums the 4 partials per batch and adds
bo + bv @ Wo^T (the V-bias contribution is exactly a constant row after
softmax normalization, so it never touches the device).

Layouts (per core, all matmul operands at partition base 0):
  qt  [128, pair, S]   q^T, two heads stacked on partitions (d dims)
  ktz [128, head, S]   k^T zero-padded: even heads live on partitions
                       0-63 (64-127 zero), odd heads on 64-127 — the
                       scores matmul is then a plain K=128 matmul
                       against the pair-stacked qt.
  v2  [128, head, kj, 65]  V blocks with an appended ones column
                       (produces the softmax denominator for free).
  scores^T [sk, sq] in PSUM -> exp on ScalarE -> bf16 tiles ->
  attnU^T [65, sq] accumulated with V2 stationary (N=512 moving), so
  no transposes are needed before the O-projection; rowsum = row 64.
  Reciprocal of the rowsum row via DVE reciprocal_approx_fast,
  replicated across partitions with gpsimd partition_broadcast.

Scheduling: engine queues are in-order, so the emission order below is
a hand software-pipelined schedule: scores are emitted one step ahead
of the attnU consumers, and independent projection / output-projection
units are interleaved as fillers one chunk ahead of where their
results are needed.  All input DMAs are consolidated into one
descriptor per (tensor, seq-chunk) and issued from the Sync queue so
the ACT queue only ever runs exp.
"""

import numpy as np
import ml_dtypes

import concourse.bass as bass
import concourse.bacc as bacc
import concourse.tile as tile
import concourse.mybir as mybir
from concourse.bass_utils import run_bass_kernel_spmd

BF16 = mybir.dt.bfloat16
F32 = mybir.dt.float32
NPBF16 = ml_dtypes.bfloat16
AF = mybir.ActivationFunctionType

B = 2
S = 2048
D = 1024
H = 16
HD = 64
NCORES = 8
HPC = 4            # heads per core
NPAIRS = 2         # head pairs per core
NQ = S // 128      # 16 query/key blocks of 128
QCH = 512          # sq chunk width
NCH = S // QCH     # 4 chunks
KT = D // 128      # 8 contraction tiles for projections


def _emit(tc: tile.TileContext, io: dict):
    nc = tc.nc

    persist = tc.alloc_tile_pool(name="persist", bufs=1)

    # ---- constants ----
    ones128 = persist.tile([128, 128], BF16, name="ones128")
    nc.gpsimd.memset(ones128, 1.0)

    # ---- persistent SBUF arrays ----
    qt = persist.tile([128, NPAIRS, S], BF16, name="qt")
    ktz = persist.tile([128, HPC, S], BF16, name="ktz")
    v2 = persist.tile([128, HPC, NQ, 65], BF16, name="v2")
    fs = persist.tile([128, HPC, NQ, 65], BF16, name="fs")
    att = persist.tile([128, NPAIRS, S], BF16, name="att")

    qts = persist.tile([128, KT, S], BF16, name="qts")
    kts = persist.tile([128, KT, S], BF16, name="kts")
    vts = persist.tile([128, KT, S], BF16, name="vts")
    wqt = persist.tile([128, KT, 256], BF16, name="wqt")
    wkt = persist.tile([128, KT, 256], BF16, name="wkt")
    wvt = persist.tile([128, KT, 256], BF16, name="wvt")
    wot = persist.tile([128, NPAIRS, D], BF16, name="wot")
    bqc = persist.tile([128, NPAIRS], F32, name="bqc")
    bkc = persist.tile([128, NPAIRS], F32, name="bkc")

    # ---- input DMAs: one descriptor per (tensor, chunk), all on Sync so
    # no compute queue ever stalls behind a descriptor issue.  Order is
    # the priority order of first use.
    dma = nc.sync
    dma.dma_start(bqc, io["BQC"])
    dma.dma_start(bkc, io["BKC"])
    dma.dma_start(wqt, io["WQT"])
    dma.dma_start(qts[:, :, 0:QCH], io["QT"][:, :, 0:QCH])
    dma.dma_start(wkt, io["WKT"])
    dma.dma_start(kts[:, :, 0:QCH], io["KT"][:, :, 0:QCH])
    dma.dma_start(wvt, io["WVT"])
    dma.dma_start(vts[:, :, 0:QCH], io["VT"][:, :, 0:QCH])
    dma.dma_start(vts[:, :, QCH:2 * QCH], io["VT"][:, :, QCH:2 * QCH])
    dma.dma_start(qts[:, :, QCH:2 * QCH], io["QT"][:, :, QCH:2 * QCH])
    dma.dma_start(kts[:, :, QCH:2 * QCH], io["KT"][:, :, QCH:2 * QCH])
    dma.dma_start(vts[:, :, 2 * QCH:3 * QCH], io["VT"][:, :, 2 * QCH:3 * QCH])
    dma.dma_start(vts[:, :, 3 * QCH:], io["VT"][:, :, 3 * QCH:])
    dma.dma_start(wot, io["WOT"])
    dma.dma_start(qts[:, :, 2 * QCH:3 * QCH], io["QT"][:, :, 2 * QCH:3 * QCH])
    dma.dma_start(kts[:, :, 2 * QCH:3 * QCH], io["KT"][:, :, 2 * QCH:3 * QCH])
    dma.dma_start(qts[:, :, 3 * QCH:], io["QT"][:, :, 3 * QCH:])
    dma.dma_start(kts[:, :, 3 * QCH:], io["KT"][:, :, 3 * QCH:])

    for h in range(HPC):  # zero the unused half of each ktz head
        half = slice(64, 128) if h % 2 == 0 else slice(0, 64)
        nc.vector.memset(ktz[half, h, :], 0.0)
    nc.gpsimd.memset(v2[:, :, :, 64:65], 1.0)  # ones column

    pb_s = tc.alloc_tile_pool(name="pb_scores", bufs=2, space="PSUM")
    pb_a = tc.alloc_tile_pool(name="pb_attnu", bufs=2, space="PSUM")
    pb_e = tc.alloc_tile_pool(name="pb_exp", bufs=7)
    pb_r = tc.alloc_tile_pool(name="pb_recip", bufs=2)

    def vproj_unit(st):
        """V projection for key block st -> v2 tiles (no bias: folded
        into the host-side bv @ Wo^T add)."""
        psv_t = pb_s.tile([128, 2, QCH], F32, tag="sps", name=f"ps_v{st}")
        ps_v = psv_t[:, 0, 0:256]
        for t in range(KT):
            nc.tensor.matmul(ps_v, vts[:, t, st * 128:(st + 1) * 128],
                             wvt[:, t, :], start=(t == 0), stop=(t == KT - 1))
        nc.vector.tensor_copy(v2[:, :, st, 0:64],
                              ps_v.rearrange("p (h d) -> p h d", h=HPC))

    def qproj_unit(c, p):
        sq = slice(c * QCH, (c + 1) * QCH)
        psq_t = pb_s.tile([128, 2, QCH], F32, tag="sps", name=f"ps_q{p}_{c}")
        ps_q = psq_t[:, 0, :]
        for t in range(KT):
            nc.tensor.matmul(ps_q, wqt[:, t, p * 128:(p + 1) * 128],
                             qts[:, t, sq], start=(t == 0), stop=(t == KT - 1))
        nc.vector.tensor_scalar_add(qt[:, p, sq], ps_q, bqc[:, p:p + 1])

    def kproj_unit(c, p):
        sq = slice(c * QCH, (c + 1) * QCH)
        psk_t = pb_s.tile([128, 2, QCH], F32, tag="sps", name=f"ps_k{p}_{c}")
        ps_k = psk_t[:, 0, :]
        for t in range(KT):
            nc.tensor.matmul(ps_k, wkt[:, t, p * 128:(p + 1) * 128],
                             kts[:, t, sq], start=(t == 0), stop=(t == KT - 1))
        nc.vector.tensor_scalar_add(ktz[0:64, 2 * p, sq], ps_k[0:64, :],
                                    bkc[0:64, p:p + 1])
        nc.vector.tensor_scalar_add(ktz[64:128, 2 * p + 1, sq],
                                    ps_k[64:128, :], bkc[64:128, p:p + 1])

    def folded_suffixes():
        nc.vector.memset(fs[:, :, NQ - 1, :], 0.0)
        for h in range(HPC):
            for q in range(NQ - 2, -1, -1):
                nc.vector.tensor_add(fs[:, h, q, :], fs[:, h, q + 1, :],
                                     v2[:, h, q + 1, :])

    aups_tiles = {}

    def score_step(c, p, kj):
        """scores matmuls + exp (+ diagonal fill) for one (c, p, kj)."""
        c0 = max(kj - 4 * c, 0) * 128
        sps = pb_s.tile([128, 2, QCH], F32, tag="sps", name=f"sps{p}_{c}_{kj}")
        for hl in range(2):
            nc.tensor.matmul(
                sps[:, hl, c0:QCH],
                ktz[:, 2 * p + hl, kj * 128:(kj + 1) * 128],
                qt[:, p, c * QCH + c0:(c + 1) * QCH],
                start=True, stop=True)
        ext = pb_e.tile([128, 2, QCH], BF16, tag="ext",
                        name=f"ext{p}_{c}_{kj}")
        nc.scalar.activation(ext[:, :, c0:QCH], sps[:, :, c0:QCH],
                             AF.Exp, scale=0.125)
        if kj >= 4 * c:  # diagonal block: masked exp entries -> 1.0
            for hl in range(2):
                nc.gpsimd.affine_select(
                    out=ext[:, hl, c0:c0 + 128],
                    in_=ext[:, hl, c0:c0 + 128],
                    compare_op=mybir.AluOpType.is_ge,
                    fill=1.0, base=0,
                    pattern=[[1, 128]], channel_multiplier=-1)
        return ext

    def attn_step(c, p, kj, ext):
        """attnU accumulation for one (c, p, kj); early FS adds at kj==0."""
        c0 = max(kj - 4 * c, 0) * 128
        if kj == 0:
            aups = pb_a.tile([65, 2, QCH], F32, tag="aups",
                             name=f"aups{p}_{c}")
            aups_tiles[(p, c)] = aups
        aups = aups_tiles[(p, c)]
        for hl in range(2):
            # masked cols < c0 get their (block-constant) contribution
            # from the FS matmuls
            nc.tensor.matmul(
                aups[:, hl, c0:QCH],
                v2[:, 2 * p + hl, kj, :],
                ext[:, hl, c0:QCH],
                start=(kj == 0),
                stop=(kj == 4 * c + 3 and c > 0))
        if kj == 0 and c > 0:
            # suffix adds commute with the accumulation: emit them up
            # front so finalize can start the moment the last attnU lands
            for hl in range(2):
                h = 2 * p + hl
                for ql in range(4):
                    qi = 4 * c + ql
                    if qi < NQ - 1:
                        nc.tensor.matmul(
                            aups[:, hl, ql * 128:(ql + 1) * 128],
                            fs[:, h, qi, :], ones128,
                            start=False, stop=False)

    def fs0_unit(p):
        """FS adds for chunk 0 (fs is not ready when chunk 0 runs)."""
        aups = aups_tiles[(p, 0)]
        for hl in range(2):
            for ql in range(4):
                nc.tensor.matmul(
                    aups[:, hl, ql * 128:(ql + 1) * 128],
                    fs[:, 2 * p + hl, ql, :], ones128,
                    start=False, stop=(ql == 3))

    def finalize(c, p):
        """rowsum reciprocal + broadcast + normalize into att."""
        ch = slice(c * QCH, (c + 1) * QCH)
        aups = aups_tiles[(p, c)]
        rec = pb_r.tile([128, 2, QCH], F32, tag="lr", name=f"rec{p}_{c}")
        nc.vector.reciprocal(rec[64:65, :, :], aups[64:65, :, :])
        rep = pb_r.tile([128, 2, QCH], F32, tag="rep", name=f"rep{p}_{c}")
        r = p * NCH + c
        nc.gpsimd.dma_start(io["dscratch"][r:r + 1, :], rec[64:65, :, :])
        nc.gpsimd.dma_start(
            rep[0:64, :, :],
            io["dscratch"][r:r + 1, :].rearrange("r (a b) -> r a b", a=2)
            .broadcast_to([64, 2, QCH]))
        for hl in range(2):
            nc.vector.tensor_mul(
                att[hl * 64:(hl + 1) * 64, p, ch],
                aups[0:64, hl, :],
                rep[0:64, hl, :])

    ob_tiles = {}

    def outproj_unit(st, dc):
        pso = pb_s.tile([128, 2, QCH], F32, tag="sps", name=f"pso{st}_{dc}")
        for p in range(NPAIRS):
            # K=128 contraction = both heads of the pair stacked
            nc.tensor.matmul(
                pso[:, 0, :],
                att[:, p, st * 128:(st + 1) * 128],
                wot[:, p, dc * 512:(dc + 1) * 512],
                start=(p == 0), stop=(p == NPAIRS - 1))
        if dc == 0:
            ob_tiles[st] = pb_e.tile([128, 2, QCH], BF16, tag="ob",
                                     name=f"ob{st}")
        ob = ob_tiles[st]
        nc.vector.tensor_copy(ob[:, dc, :], pso[:, 0, :])
        if dc == 1:
            nc.gpsimd.dma_start(io["out"][st * 128:(st + 1) * 128, :],
                                ob[:, :, :].rearrange("p a b -> p (a b)"))

    # ------------------------------------------------------------------
    # Software-pipelined emission schedule.
    # ------------------------------------------------------------------
    def chunk_steps(c):
        """Yield the pipelined S/A steps of chunk c: scores one step
        ahead of the matching attnU, pairs interleaved."""
        nkj = 4 * c + 4
        pend = []            # (c, p, kj, ext) awaiting attnU emission
        for kj in range(nkj):
            for p in range(NPAIRS):
                ext = score_step(c, p, kj)
                if pend:
                    attn_step(*pend.pop(0))
                    yield
                pend.append((c, p, kj, ext))
        while pend:
            attn_step(*pend.pop(0))
            yield

    def run_chunk(c, fillers, pre_tail=(), tail=()):
        """Emit chunk c with filler units after each pipeline step;
        leftover fillers and `pre_tail` are emitted after the chunk's
        steps but before finalize, `tail` after finalize."""
        fillers = list(fillers)
        nsteps = 8 * c + 8
        per = max(1, (len(fillers) + nsteps - 1) // nsteps) if fillers else 0
        for _ in chunk_steps(c):
            for _ in range(per):
                if fillers:
                    fillers.pop(0)()
        for f in fillers:
            f()
        for f in pre_tail:
            f()
        for p in range(NPAIRS):
            finalize(c, p)
        for f in tail:
            f()

    # startup: projections for chunk 0, V blocks 0-3
    for p in range(NPAIRS):
        qproj_unit(0, p)
        kproj_unit(0, p)
    for st in range(4):
        vproj_unit(st)

    def fs_chain():
        folded_suffixes()

    # chunk 0: fill with remaining V blocks; chunk-1 projections run
    # after the chunk while the DVE folds the suffixes, so the FS0
    # matmuls rarely stall the PE.
    run_chunk(0, fillers=[lambda st=st: vproj_unit(st) for st in range(4, NQ)],
              pre_tail=(
        [lambda: qproj_unit(1, 0), lambda: kproj_unit(1, 0), fs_chain,
         lambda: qproj_unit(1, 1), lambda: kproj_unit(1, 1)]
        + [lambda p=p: fs0_unit(p) for p in range(NPAIRS)]))

    # chunk 1: fill with chunk-2 projections
    run_chunk(1, fillers=(
        [lambda p=p: qproj_unit(2, p) for p in range(NPAIRS)]
        + [lambda p=p: kproj_unit(2, p) for p in range(NPAIRS)]))

    # chunk 2: chunk-3 projections early, then outproj of chunks 0-1
    run_chunk(2, fillers=(
        [lambda p=p: qproj_unit(3, p) for p in range(NPAIRS)]
        + [lambda p=p: kproj_unit(3, p) for p in range(NPAIRS)]
        + [lambda st=st, dc=dc: outproj_unit(st, dc)
           for st in range(0, 4) for dc in range(2)]
        + [lambda st=st, dc=dc: outproj_unit(st, dc)
           for st in range(4, 8) for dc in range(2)]))

    # chunk 3: outproj of chunk 2 as fillers; chunk-3 outproj is the tail
    run_chunk(3, fillers=(
        [lambda st=st, dc=dc: outproj_unit(st, dc)
         for st in range(8, 12) for dc in range(2)]),
        tail=[lambda st=st, dc=dc: outproj_unit(st, dc)
              for st in range(12, 16) for dc in range(2)])

    pb_r.release()
    pb_e.release()
    pb_a.release()
    pb_s.release()
    persist.release()


_CACHED = None


def _build():
    global _CACHED
    if _CACHED is not None:
        return _CACHED
    nc = bacc.Bacc("TRN2", target_bir_lowering=False, debug=False)
    io = {
        "QT": nc.dram_tensor("QT", [128, KT, S], BF16, kind="ExternalInput").ap(),
        "KT": nc.dram_tensor("KT", [128, KT, S], BF16, kind="ExternalInput").ap(),
        "VT": nc.dram_tensor("VT", [128, KT, S], BF16, kind="ExternalInput").ap(),
        "WQT": nc.dram_tensor("WQT", [128, KT, 256], BF16, kind="ExternalInput").ap(),
        "WKT": nc.dram_tensor("WKT", [128, KT, 256], BF16, kind="ExternalInput").ap(),
        "WVT": nc.dram_tensor("WVT", [128, KT, 256], BF16, kind="ExternalInput").ap(),
        "WOT": nc.dram_tensor("WOT", [128, NPAIRS, D], BF16, kind="ExternalInput").ap(),
        "BQC": nc.dram_tensor("BQC", [128, NPAIRS], F32, kind="ExternalInput").ap(),
        "BKC": nc.dram_tensor("BKC", [128, NPAIRS], F32, kind="ExternalInput").ap(),
        "out": nc.dram_tensor("out", [S, D], BF16, kind="ExternalOutput").ap(),
        "dscratch": nc.dram_tensor("dscratch", [NPAIRS * NCH, 2 * QCH], F32,
                                   kind="Internal").ap(),
    }
    with tile.TileContext(nc) as tc:
        _emit(tc, io)
    nc.compile()
    _CACHED = (nc, io)
    return _CACHED


def _tiled(a, inner):
    """[R, C] -> [128, R//128, C] with row r = t*128 + p mapped to
    [p, t, :], contiguous."""
    r, c = a.shape
    assert r % 128 == 0 and c == inner
    return np.ascontiguousarray(
        a.reshape(r // 128, 128, c).transpose(1, 0, 2))


def make_in_maps(Q, K, V, Wq, bq, Wk, bk, Wv, bv, Wo):
    """Build the 8 per-core input dicts (host-side sharding)."""
    Q = np.asarray(Q, np.float32)
    K = np.asarray(K, np.float32)
    V = np.asarray(V, np.float32)
    qt = [_tiled(np.ascontiguousarray(Q[b].T).astype(NPBF16), S) for b in range(B)]
    kt = [_tiled(np.ascontiguousarray(K[b].T).astype(NPBF16), S) for b in range(B)]
    vt = [_tiled(np.ascontiguousarray(V[b].T).astype(NPBF16), S) for b in range(B)]
    in_maps = []
    for core in range(NCORES):
        b, g = divmod(core, 4)
        rows = slice(g * 256, (g + 1) * 256)
        wq = np.ascontiguousarray(np.asarray(Wq, np.float32)[rows].T).astype(NPBF16)
        wk = np.ascontiguousarray(np.asarray(Wk, np.float32)[rows].T).astype(NPBF16)
        wv = np.ascontiguousarray(np.asarray(Wv, np.float32)[rows].T).astype(NPBF16)
        wo = np.ascontiguousarray(np.asarray(Wo, np.float32)[:, rows].T).astype(NPBF16)
        in_maps.append({
            "QT": qt[b], "KT": kt[b], "VT": vt[b],
            "WQT": _tiled(wq, 256),
            "WKT": _tiled(wk, 256),
            "WVT": _tiled(wv, 256),
            "WOT": _tiled(wo, D),
            "BQC": np.ascontiguousarray(
                np.asarray(bq, np.float32)[rows].reshape(2, 128).T),
            "BKC": np.ascontiguousarray(
                np.asarray(bk, np.float32)[rows].reshape(2, 128).T),
        })
    return in_maps


def kernel(Q, K, V, mask, Wq, bq, Wk, bk, Wv, bv, Wo, bo, _results_hook=None):
    nc, _io = _build()
    in_maps = make_in_maps(Q, K, V, Wq, bq, Wk, bk, Wv, bv, Wo)
    res = run_bass_kernel_spmd(nc, in_maps, core_ids=list(range(NCORES)))
    if _results_hook is not None:
        _results_hook(res)
    out = np.zeros((B, S, D), np.float32)
    for core in range(NCORES):
        out[core // 4] += np.asarray(res.results[core]["out"], np.float32)
    # V-bias folds to a constant row post-softmax; add it with bo here.
    out += (np.asarray(bv, np.float32) @ np.asarray(Wo, np.float32).T
            + np.asarray(bo, np.float32))
    return out


# revision 12
# speedup vs baseline: 1.1714x; 1.1714x over previous
"""Self-contained Trainium2 Bass kernel for MultiHeadAttention.

Problem: B=2, S=2048, D=1024, H=16, hd=64, with the reference's
masked_fill(mask==0, -1e-09) quirk: masked scores become ~0.0, so
exp(masked) == 1.0 in fp32 and every key position participates in the
softmax denominator. Fully-masked key blocks therefore contribute a
block-constant suffix sum of V rows, added via cheap rank-1-style
matmuls instead of full score/attn matmuls.

Sharding: 8 cores = 2 batches x 4 head-groups (4 heads per core).
Each core computes a partial [S, D] output (its 4 heads pushed through
the O-projection); the host sums the 4 partials per batch and adds
bo + bv @ Wo^T (the V-bias contribution is exactly a constant row after
softmax normalization, so it never touches the device).

Layouts (per core, all matmul operands at partition base 0):
  qt  [128, pair, S]   q^T, two heads stacked on partitions (d dims)
  ktz [128, head, S]   k^T zero-padded: even heads live on partitions
                       0-63 (64-127 zero), odd heads on 64-127 — the
                       scores matmul is then a plain K=128 matmul
                       against the pair-stacked qt.
  v2  [128, head, kj, 65]  V blocks with an appended ones column
                       (produces the softmax denominator for free).
  scores^T [sk, sq] in PSUM -> exp on ScalarE -> bf16 tiles ->
  attnU^T [65, sq] accumulated with V2 stationary (N=512 moving), so
  no transposes are needed before the O-projection; rowsum = row 64.
  Reciprocal of the rowsum row via DVE reciprocal_approx_fast,
  replicated across partitions with gpsimd partition_broadcast.

Scheduling: engine queues are in-order, so the emission order below is
a hand software-pipelined schedule: scores are emitted one step ahead
of the attnU consumers, and independent projection / output-projection
units are interleaved as fillers one chunk ahead of where their
results are needed.  All input DMAs are consolidated into one
descriptor per (tensor, seq-chunk) and issued from the Sync queue so
the ACT queue only ever runs exp.
"""

import numpy as np
import ml_dtypes

import concourse.bass as bass
import concourse.bacc as bacc
import concourse.tile as tile
import concourse.mybir as mybir
from concourse.bass_utils import run_bass_kernel_spmd

BF16 = mybir.dt.bfloat16
F32 = mybir.dt.float32
NPBF16 = ml_dtypes.bfloat16
AF = mybir.ActivationFunctionType

B = 2
S = 2048
D = 1024
H = 16
HD = 64
NCORES = 8
HPC = 4            # heads per core
NPAIRS = 2         # head pairs per core
NQ = S // 128      # 16 query/key blocks of 128
QCH = 512          # sq chunk width
NCH = S // QCH     # 4 chunks
KT = D // 128      # 8 contraction tiles for projections


def _emit(tc: tile.TileContext, io: dict):
    nc = tc.nc

    persist = tc.alloc_tile_pool(name="persist", bufs=1)

    # ---- constants ----
    ones128 = persist.tile([128, 128], BF16, name="ones128")
    nc.gpsimd.memset(ones128, 1.0)

    # ---- persistent SBUF arrays ----
    qt = persist.tile([128, NPAIRS, S], BF16, name="qt")
    ktz = persist.tile([128, HPC, S], BF16, name="ktz")
    v2 = persist.tile([128, HPC, NQ, 65], BF16, name="v2")
    fs = persist.tile([128, HPC, NQ, 65], BF16, name="fs")
    att = persist.tile([128, NPAIRS, S], BF16, name="att")

    qts = persist.tile([128, KT, S], BF16, name="qts")
    kts = persist.tile([128, KT, S], BF16, name="kts")
    vts = persist.tile([128, KT, S], BF16, name="vts")
    wqt = persist.tile([128, KT, 256], BF16, name="wqt")
    wkt = persist.tile([128, KT, 256], BF16, name="wkt")
    wvt = persist.tile([128, KT, 256], BF16, name="wvt")
    wot = persist.tile([128, NPAIRS, D], BF16, name="wot")
    bqc = persist.tile([128, NPAIRS], F32, name="bqc")
    bkc = persist.tile([128, NPAIRS], F32, name="bkc")

    # ---- input DMAs: one descriptor per (tensor, chunk), all on Sync so
    # no compute queue ever stalls behind a descriptor issue.  Order is
    # the priority order of first use.
    dma = nc.sync
    dma.dma_start(bqc, io["BQC"])
    dma.dma_start(bkc, io["BKC"])
    dma.dma_start(wqt, io["WQT"])
    dma.dma_start(qts[:, :, 0:QCH], io["QT"][:, :, 0:QCH])
    dma.dma_start(wkt, io["WKT"])
    dma.dma_start(kts[:, :, 0:QCH], io["KT"][:, :, 0:QCH])
    dma.dma_start(wvt, io["WVT"])
    dma.dma_start(vts[:, :, 0:QCH], io["VT"][:, :, 0:QCH])
    dma.dma_start(vts[:, :, QCH:2 * QCH], io["VT"][:, :, QCH:2 * QCH])
    dma.dma_start(qts[:, :, QCH:2 * QCH], io["QT"][:, :, QCH:2 * QCH])
    dma.dma_start(kts[:, :, QCH:2 * QCH], io["KT"][:, :, QCH:2 * QCH])
    dma.dma_start(vts[:, :, 2 * QCH:3 * QCH], io["VT"][:, :, 2 * QCH:3 * QCH])
    dma.dma_start(vts[:, :, 3 * QCH:], io["VT"][:, :, 3 * QCH:])
    dma.dma_start(wot, io["WOT"])
    dma.dma_start(qts[:, :, 2 * QCH:3 * QCH], io["QT"][:, :, 2 * QCH:3 * QCH])
    dma.dma_start(kts[:, :, 2 * QCH:3 * QCH], io["KT"][:, :, 2 * QCH:3 * QCH])
    dma.dma_start(qts[:, :, 3 * QCH:], io["QT"][:, :, 3 * QCH:])
    dma.dma_start(kts[:, :, 3 * QCH:], io["KT"][:, :, 3 * QCH:])

    for h in range(HPC):  # zero the unused half of each ktz head
        half = slice(64, 128) if h % 2 == 0 else slice(0, 64)
        nc.vector.memset(ktz[half, h, :], 0.0)
    nc.gpsimd.memset(v2[:, :, :, 64:65], 1.0)  # ones column

    pb_s = tc.alloc_tile_pool(name="pb_scores", bufs=2, space="PSUM")
    pb_a = tc.alloc_tile_pool(name="pb_attnu", bufs=2, space="PSUM")
    pb_e = tc.alloc_tile_pool(name="pb_exp", bufs=7)
    pb_r = tc.alloc_tile_pool(name="pb_recip", bufs=2)

    def vproj_unit(st):
        """V projection for key block st -> v2 tiles (no bias: folded
        into the host-side bv @ Wo^T add)."""
        psv_t = pb_s.tile([128, 2, QCH], F32, tag="sps", name=f"ps_v{st}")
        ps_v = psv_t[:, 0, 0:256]
        for t in range(KT):
            nc.tensor.matmul(ps_v, vts[:, t, st * 128:(st + 1) * 128],
                             wvt[:, t, :], start=(t == 0), stop=(t == KT - 1))
        nc.vector.tensor_copy(v2[:, :, st, 0:64],
                              ps_v.rearrange("p (h d) -> p h d", h=HPC))

    def qproj_unit(c, p):
        sq = slice(c * QCH, (c + 1) * QCH)
        psq_t = pb_s.tile([128, 2, QCH], F32, tag="sps", name=f"ps_q{p}_{c}")
        ps_q = psq_t[:, 0, :]
        for t in range(KT):
            nc.tensor.matmul(ps_q, wqt[:, t, p * 128:(p + 1) * 128],
                             qts[:, t, sq], start=(t == 0), stop=(t == KT - 1))
        nc.vector.tensor_scalar_add(qt[:, p, sq], ps_q, bqc[:, p:p + 1])

    def kproj_unit(c, p):
        sq = slice(c * QCH, (c + 1) * QCH)
        psk_t = pb_s.tile([128, 2, QCH], F32, tag="sps", name=f"ps_k{p}_{c}")
        ps_k = psk_t[:, 0, :]
        for t in range(KT):
            nc.tensor.matmul(ps_k, wkt[:, t, p * 128:(p + 1) * 128],
                             kts[:, t, sq], start=(t == 0), stop=(t == KT - 1))
        nc.vector.tensor_scalar_add(ktz[0:64, 2 * p, sq], ps_k[0:64, :],
                                    bkc[0:64, p:p + 1])
        nc.vector.tensor_scalar_add(ktz[64:128, 2 * p + 1, sq],
                                    ps_k[64:128, :], bkc[64:128, p:p + 1])

    def folded_suffixes():
        nc.vector.memset(fs[:, :, NQ - 1, :], 0.0)
        for h in range(HPC):
            for q in range(NQ - 2, -1, -1):
                nc.vector.tensor_add(fs[:, h, q, :], fs[:, h, q + 1, :],
                                     v2[:, h, q + 1, :])

    aups_tiles = {}

    def score_step(c, p, kj):
        """scores matmuls + exp (+ diagonal fill) for one (c, p, kj)."""
        c0 = max(kj - 4 * c, 0) * 128
        sps = pb_s.tile([128, 2, QCH], F32, tag="sps", name=f"sps{p}_{c}_{kj}")
        for hl in range(2):
            nc.tensor.matmul(
                sps[:, hl, c0:QCH],
                ktz[:, 2 * p + hl, kj * 128:(kj + 1) * 128],
                qt[:, p, c * QCH + c0:(c + 1) * QCH],
                start=True, stop=True)
        ext = pb_e.tile([128, 2, QCH], BF16, tag="ext",
                        name=f"ext{p}_{c}_{kj}")
        nc.scalar.activation(ext[:, :, c0:QCH], sps[:, :, c0:QCH],
                             AF.Exp, scale=0.125)
        if kj >= 4 * c:  # diagonal block: masked exp entries -> 1.0
            for hl in range(2):
                nc.gpsimd.affine_select(
                    out=ext[:, hl, c0:c0 + 128],
                    in_=ext[:, hl, c0:c0 + 128],
                    compare_op=mybir.AluOpType.is_ge,
                    fill=1.0, base=0,
                    pattern=[[1, 128]], channel_multiplier=-1)
        return ext

    def attn_step(c, p, kj, ext):
        """attnU accumulation for one (c, p, kj); early FS adds at kj==0."""
        c0 = max(kj - 4 * c, 0) * 128
        if kj == 0:
            aups = pb_a.tile([65, 2, QCH], F32, tag="aups",
                             name=f"aups{p}_{c}")
            aups_tiles[(p, c)] = aups
        aups = aups_tiles[(p, c)]
        for hl in range(2):
            # masked cols < c0 get their (block-constant) contribution
            # from the FS matmuls
            nc.tensor.matmul(
                aups[:, hl, c0:QCH],
                v2[:, 2 * p + hl, kj, :],
                ext[:, hl, c0:QCH],
                start=(kj == 0),
                stop=(kj == 4 * c + 3 and c > 0))
        if kj == 0 and c > 0:
            # suffix adds commute with the accumulation: emit them up
            # front so finalize can start the moment the last attnU lands
            for hl in range(2):
                h = 2 * p + hl
                for ql in range(4):
                    qi = 4 * c + ql
                    if qi < NQ - 1:
                        nc.tensor.matmul(
                            aups[:, hl, ql * 128:(ql + 1) * 128],
                            fs[:, h, qi, :], ones128,
                            start=False, stop=False)

    def fs0_unit(p):
        """FS adds for chunk 0 (fs is not ready when chunk 0 runs)."""
        aups = aups_tiles[(p, 0)]
        for hl in range(2):
            for ql in range(4):
                nc.tensor.matmul(
                    aups[:, hl, ql * 128:(ql + 1) * 128],
                    fs[:, 2 * p + hl, ql, :], ones128,
                    start=False, stop=(ql == 3))

    def finalize(c, p):
        """rowsum reciprocal + broadcast + normalize into att."""
        ch = slice(c * QCH, (c + 1) * QCH)
        aups = aups_tiles[(p, c)]
        lnr = pb_r.tile([128, 2, QCH], F32, tag="lr", name=f"lnr{p}_{c}")
        nc.scalar.activation(lnr[64:65, :, :], aups[64:65, :, :], AF.Ln)
        rec = pb_r.tile([128, 2, QCH], F32, tag="lr", name=f"rec{p}_{c}")
        nc.scalar.activation(rec[64:65, :, :], lnr[64:65, :, :], AF.Exp,
                             scale=-1.0)
        rep = pb_r.tile([128, 2, QCH], F32, tag="rep", name=f"rep{p}_{c}")
        r = p * NCH + c
        nc.gpsimd.dma_start(io["dscratch"][r:r + 1, :], rec[64:65, :, :])
        nc.gpsimd.dma_start(
            rep[0:64, :, :],
            io["dscratch"][r:r + 1, :].rearrange("r (a b) -> r a b", a=2)
            .broadcast_to([64, 2, QCH]))
        for hl in range(2):
            nc.vector.tensor_mul(
                att[hl * 64:(hl + 1) * 64, p, ch],
                aups[0:64, hl, :],
                rep[0:64, hl, :])

    ob_tiles = {}

    def outproj_unit(st, dc):
        pso = pb_s.tile([128, 2, QCH], F32, tag="sps", name=f"pso{st}_{dc}")
        for p in range(NPAIRS):
            # K=128 contraction = both heads of the pair stacked
            nc.tensor.matmul(
                pso[:, 0, :],
                att[:, p, st * 128:(st + 1) * 128],
                wot[:, p, dc * 512:(dc + 1) * 512],
                start=(p == 0), stop=(p == NPAIRS - 1))
        if dc == 0:
            ob_tiles[st] = pb_e.tile([128, 2, QCH], BF16, tag="ob",
                                     name=f"ob{st}")
        ob = ob_tiles[st]
        nc.vector.tensor_copy(ob[:, dc, :], pso[:, 0, :])
        if dc == 1:
            nc.gpsimd.dma_start(io["out"][st * 128:(st + 1) * 128, :],
                                ob[:, :, :].rearrange("p a b -> p (a b)"))

    # ------------------------------------------------------------------
    # Software-pipelined emission schedule.
    # ------------------------------------------------------------------
    def chunk_steps(c):
        """Yield the pipelined S/A steps of chunk c: scores one step
        ahead of the matching attnU, pairs interleaved."""
        nkj = 4 * c + 4
        pend = []            # (c, p, kj, ext) awaiting attnU emission
        for kj in range(nkj):
            for p in range(NPAIRS):
                ext = score_step(c, p, kj)
                if pend:
                    attn_step(*pend.pop(0))
                    yield
                pend.append((c, p, kj, ext))
        while pend:
            attn_step(*pend.pop(0))
            yield

    def run_chunk(c, fillers, pre_tail=(), tail=()):
        """Emit chunk c with filler units after each pipeline step;
        leftover fillers and `pre_tail` are emitted after the chunk's
        steps but before finalize, `tail` after finalize."""
        fillers = list(fillers)
        nsteps = 8 * c + 8
        per = max(1, (len(fillers) + nsteps - 1) // nsteps) if fillers else 0
        for _ in chunk_steps(c):
            for _ in range(per):
                if fillers:
                    fillers.pop(0)()
        for f in fillers:
            f()
        for f in pre_tail:
            f()
        for p in range(NPAIRS):
            finalize(c, p)
        for f in tail:
            f()

    # startup: projections for chunk 0, V blocks 0-3
    for p in range(NPAIRS):
        qproj_unit(0, p)
        kproj_unit(0, p)
    for st in range(4):
        vproj_unit(st)

    def fs_chain():
        folded_suffixes()

    # chunk 0: fill with remaining V blocks; chunk-1 projections run
    # after the chunk while the DVE folds the suffixes, so the FS0
    # matmuls rarely stall the PE.
    run_chunk(0, fillers=[lambda st=st: vproj_unit(st) for st in range(4, NQ)],
              pre_tail=(
        [lambda: qproj_unit(1, 0), lambda: kproj_unit(1, 0), fs_chain,
         lambda: qproj_unit(1, 1), lambda: kproj_unit(1, 1)]
        + [lambda p=p: fs0_unit(p) for p in range(NPAIRS)]))

    # chunk 1: fill with chunk-2 projections
    run_chunk(1, fillers=(
        [lambda p=p: qproj_unit(2, p) for p in range(NPAIRS)]
        + [lambda p=p: kproj_unit(2, p) for p in range(NPAIRS)]))

    # chunk 2: chunk-3 projections early, then outproj of chunks 0-1
    run_chunk(2, fillers=(
        [lambda p=p: qproj_unit(3, p) for p in range(NPAIRS)]
        + [lambda p=p: kproj_unit(3, p) for p in range(NPAIRS)]
        + [lambda st=st, dc=dc: outproj_unit(st, dc)
           for st in range(0, 4) for dc in range(2)]
        + [lambda st=st, dc=dc: outproj_unit(st, dc)
           for st in range(4, 8) for dc in range(2)]))

    # chunk 3: outproj of chunk 2 as fillers; chunk-3 outproj is the tail
    run_chunk(3, fillers=(
        [lambda st=st, dc=dc: outproj_unit(st, dc)
         for st in range(8, 12) for dc in range(2)]),
        tail=[lambda st=st, dc=dc: outproj_unit(st, dc)
              for st in range(12, 16) for dc in range(2)])

    pb_r.release()
    pb_e.release()
    pb_a.release()
    pb_s.release()
    persist.release()


_CACHED = None


def _patch_act_tables():
    """Make Exp and Ln resolve to the single combined table set so the
    per-chunk recip (Ln/Exp) doesn't thrash ACT_TABLE_LOADs against the
    softmax Exp calls. Set positions (= act_func_set_id) are preserved;
    only membership of Exp/Ln in other sets is hidden from the selector."""
    from concourse import hw_specs
    orig = hw_specs.get_activation_tables

    def patched(arch):
        t = dict(orig(arch))
        if "natural_log_exp_and_others" in t:
            for name in t:
                if name != "natural_log_exp_and_others":
                    t[name] = t[name] - {AF.Exp, AF.Ln}
        return t

    bacc.get_activation_tables = patched


def _build():
    global _CACHED
    if _CACHED is not None:
        return _CACHED
    _patch_act_tables()
    nc = bacc.Bacc("TRN2", target_bir_lowering=False, debug=False)
    io = {
        "QT": nc.dram_tensor("QT", [128, KT, S], BF16, kind="ExternalInput").ap(),
        "KT": nc.dram_tensor("KT", [128, KT, S], BF16, kind="ExternalInput").ap(),
        "VT": nc.dram_tensor("VT", [128, KT, S], BF16, kind="ExternalInput").ap(),
        "WQT": nc.dram_tensor("WQT", [128, KT, 256], BF16, kind="ExternalInput").ap(),
        "WKT": nc.dram_tensor("WKT", [128, KT, 256], BF16, kind="ExternalInput").ap(),
        "WVT": nc.dram_tensor("WVT", [128, KT, 256], BF16, kind="ExternalInput").ap(),
        "WOT": nc.dram_tensor("WOT", [128, NPAIRS, D], BF16, kind="ExternalInput").ap(),
        "BQC": nc.dram_tensor("BQC", [128, NPAIRS], F32, kind="ExternalInput").ap(),
        "BKC": nc.dram_tensor("BKC", [128, NPAIRS], F32, kind="ExternalInput").ap(),
        "out": nc.dram_tensor("out", [S, D], BF16, kind="ExternalOutput").ap(),
        "dscratch": nc.dram_tensor("dscratch", [NPAIRS * NCH, 2 * QCH], F32,
                                   kind="Internal").ap(),
    }
    with tile.TileContext(nc) as tc:
        _emit(tc, io)
    nc.compile()
    _CACHED = (nc, io)
    return _CACHED


def _tiled(a, inner):
    """[R, C] -> [128, R//128, C] with row r = t*128 + p mapped to
    [p, t, :], contiguous."""
    r, c = a.shape
    assert r % 128 == 0 and c == inner
    return np.ascontiguousarray(
        a.reshape(r // 128, 128, c).transpose(1, 0, 2))


def make_in_maps(Q, K, V, Wq, bq, Wk, bk, Wv, bv, Wo):
    """Build the 8 per-core input dicts (host-side sharding)."""
    Q = np.asarray(Q, np.float32)
    K = np.asarray(K, np.float32)
    V = np.asarray(V, np.float32)
    qt = [_tiled(np.ascontiguousarray(Q[b].T).astype(NPBF16), S) for b in range(B)]
    kt = [_tiled(np.ascontiguousarray(K[b].T).astype(NPBF16), S) for b in range(B)]
    vt = [_tiled(np.ascontiguousarray(V[b].T).astype(NPBF16), S) for b in range(B)]
    in_maps = []
    for core in range(NCORES):
        b, g = divmod(core, 4)
        rows = slice(g * 256, (g + 1) * 256)
        wq = np.ascontiguousarray(np.asarray(Wq, np.float32)[rows].T).astype(NPBF16)
        wk = np.ascontiguousarray(np.asarray(Wk, np.float32)[rows].T).astype(NPBF16)
        wv = np.ascontiguousarray(np.asarray(Wv, np.float32)[rows].T).astype(NPBF16)
        wo = np.ascontiguousarray(np.asarray(Wo, np.float32)[:, rows].T).astype(NPBF16)
        in_maps.append({
            "QT": qt[b], "KT": kt[b], "VT": vt[b],
            "WQT": _tiled(wq, 256),
            "WKT": _tiled(wk, 256),
            "WVT": _tiled(wv, 256),
            "WOT": _tiled(wo, D),
            "BQC": np.ascontiguousarray(
                np.asarray(bq, np.float32)[rows].reshape(2, 128).T),
            "BKC": np.ascontiguousarray(
                np.asarray(bk, np.float32)[rows].reshape(2, 128).T),
        })
    return in_maps


def kernel(Q, K, V, mask, Wq, bq, Wk, bk, Wv, bv, Wo, bo, _results_hook=None):
    nc, _io = _build()
    in_maps = make_in_maps(Q, K, V, Wq, bq, Wk, bk, Wv, bv, Wo)
    res = run_bass_kernel_spmd(nc, in_maps, core_ids=list(range(NCORES)))
    if _results_hook is not None:
        _results_hook(res)
    out = np.zeros((B, S, D), np.float32)
    for core in range(NCORES):
        out[core // 4] += np.asarray(res.results[core]["out"], np.float32)
    # V-bias folds to a constant row post-softmax; add it with bo here.
    out += (np.asarray(bv, np.float32) @ np.asarray(Wo, np.float32).T
            + np.asarray(bo, np.float32))
    return out
